# revision 44
# baseline (speedup 1.0000x reference)
"""Multi-head causal self-attention on 8 Trainium2 NeuronCores.

Sharding: core c -> batch b = c // 2, heads 4*(c % 2) .. +4  (data parallel on
B, tensor parallel on heads).  Each core computes its 4 heads' attention for
its batch plus the partial out-projection; the host sums the two partials per
batch and adds b_out.

Per-core layout:
  xT   [128, 4, T]  x[b] transposed on host (bf16), partition-major
  qT/kT [128, 2, T] head-major: partitions = 2 heads x 64, 2 m-tiles
  v    [128, 16, 260] natural [T, hd] per head + a ones column (gives the
                    softmax denominator for free during the AV matmul)
  scores are computed transposed: sT[k, q] = kT.T @ q, the two heads of a
  pair run as row-tiled CONCURRENT matmuls (tile rows 0-63 / 64-127) into
  one [128, 2, 512] PSUM tile.

The whole kernel is emitted as one interleaved PE stream so the PE never
idles long enough for the HAM clock gate to re-throttle:
  v(0:4) qkv(b0) u(0,0) u(0,1) qkv(b1) v(4:8) u(1,0) u(1,1)
  qkv(b2) v(8:12) op(0) u(2,0) u(2,1) qkv(b3) v(12:16) op(1)
  u(3,0) u(3,1) op(2) op(3)
where u(qb,hp) is a 512-query attention unit and op(qb) the out-projection
rows qb*512..+512 (emitted once both hp units' normalized outputs exist).

Exp evacuation of score PSUM is column-split between ACT (exact exp) and
DVE (Schraudolph exp2 bit-trick: tensor_scalar -> int16 -> bf16 bitcast,
~3% per element, cancels in softmax).  Causal masking of diagonal blocks
is FUSED into the DVE evacuation: scalar_tensor_tensor adds a per-element
constant (FE_B on allowed entries, MASK_FILL on disallowed) so masked
entries bitcast to ~-3e-36 ~ -0.0 -- no separate mask multiply, and safe
whether the f32->i16 conversion saturates or wraps (it stays in range).

The AV matmuls lag the score matmuls by one key-tile step (the PE queue is
in-order; the lag hides the exp latency).  AV accumulator PSUM sets
alternate per unit.  Normalization reads the AV PSUM directly (frees the
bank at the last read): reciprocal of just the denominator row, DMA hop to
partition 0, gpsimd partition-broadcast, one multiply per head.
"""

import os
import sys
from contextlib import ExitStack

import numpy as np

for _p in ("/opt/trn_rl_repo", "/opt/pypackages"):
    if os.path.isdir(_p) and _p not in sys.path:
        sys.path.append(_p)

import concourse.bass as bass
from concourse import bacc
import concourse.mybir as mybir
import concourse.tile as tile
from concourse.bass_utils import run_bass_kernel_spmd


B, T, D = 4, 2048, 512
H, HD = 8, 64
HPC = 4  # heads per core
P = 128
KT = D // P  # k-tiles over the model dim
QB = 512  # query-unit width / psum bank width
NKT = T // P  # key tiles
NU = T // QB  # query blocks
VW = HD + 1  # v columns per head incl. the ones column

F32 = mybir.dt.float32
I16 = mybir.dt.int16
BF16 = mybir.dt.bfloat16
MMDT = BF16
EXP = mybir.ActivationFunctionType.Exp
IDENT = mybir.ActivationFunctionType.Identity

# fast-exp constants: exp(s/8) ~= bitcast_bf16(int16(s * FE_A + FE_B))
_LOG2E = 1.4426950408889634
FE_A = _LOG2E * 128.0 / 8.0
FE_B = 127.0 * 128.0 - 5.6
# masked entries: s*FE_A + MASK_FILL stays in [-32768, -31000] for |s|<=24,
# whose int16 bit patterns read as bf16 denormals ~ -3e-36 ~ zero.
MASK_FILL = -32214.0

# column where the ACT/DVE split of a full key-tile's exp evacuation sits
CA = 288  # ACT gets [0, CA), DVE gets [CA, 512)

NORM_OLD = os.environ.get("NORM_OLD", "0") == "1"
EXP_OLD = os.environ.get("EXP_OLD", "0") == "1"

try:
    import ml_dtypes
    _NP_MMDT = np.dtype(ml_dtypes.bfloat16)
except ImportError:
    _NP_MMDT = np.float32


def build_bass():
    nc = bacc.Bacc()
    xT = nc.declare_dram_parameter("xT", [P, KT, T], MMDT, isOutput=False)
    wqa = nc.declare_dram_parameter("wqa", [P, KT, 2 * P], MMDT, isOutput=False)
    wka = nc.declare_dram_parameter("wka", [P, KT, 2 * P], MMDT, isOutput=False)
    # q/k biases, laid out [channel % 128, channel // 128] for ACT bias APs
    wqkb = nc.declare_dram_parameter("wqkb", [P, 4], F32, isOutput=False)
    wva = nc.declare_dram_parameter("wva", [P, KT, HPC * VW], MMDT, isOutput=False)
    wo = nc.declare_dram_parameter("wo", [P, 2, D], MMDT, isOutput=False)
    y = nc.declare_dram_parameter("y", [T, D], BF16, isOutput=True)

    with tile.TileContext(nc) as tc, ExitStack() as ctx:
        consts = ctx.enter_context(tc.tile_pool(name="consts", bufs=1))
        qkv = ctx.enter_context(tc.tile_pool(name="qkv", bufs=1))
        attn = ctx.enter_context(tc.tile_pool(name="attn", bufs=1))
        etp = ctx.enter_context(tc.tile_pool(name="etp", bufs=4))
        nrm = ctx.enter_context(tc.tile_pool(name="nrm", bufs=3))
        yevac = ctx.enter_context(tc.tile_pool(name="yevac", bufs=3))
        # PSUM: "mm" 2 bufs x 4KB/partition (2 banks each) = 4 banks;
        # o{i}{s} 4 x [128,512]f32 (1 bank each) = 4 banks.
        mmps = ctx.enter_context(tc.tile_pool(name="mmps", bufs=2, space="PSUM"))
        aps = ctx.enter_context(tc.tile_pool(name="aps", bufs=1, space="PSUM"))

        # ---- SBUF destinations
        x_sb = consts.tile([P, KT, T], MMDT)
        wq_sb = consts.tile([P, KT, 2 * P], MMDT)
        wk_sb = consts.tile([P, KT, 2 * P], MMDT)
        wv_sb = consts.tile([P, KT, HPC * VW], MMDT)
        wqkb_sb = consts.tile([P, 4], F32)
        wo_sb = consts.tile([P, 2, D], MMDT)

        # ---- input DMA, ordered so the earliest compute's operands land
        # first; round-robin across the three DMA-capable engine queues but
        # keep the scalar engine light early (it starts evacuating PSUM soon)
        dmae = [nc.sync, nc.gpsimd, nc.scalar]
        rr = [0]

        def dma_rr(out, in_):
            dmae[rr[0] % 3].dma_start(out=out, in_=in_)
            rr[0] += 1

        # v projection runs first: it needs wv + x column block 0.  x lands
        # via sync+gpsimd queues; the scalar queue takes the weights (it has
        # no compute role until the first PSUM evacuations).
        for kt in range(KT):
            dmae[kt % 2].dma_start(
                out=x_sb[:, kt, 0:QB], in_=xT[:, kt, 0:QB]
            )
        for kt in range(KT):
            nc.scalar.dma_start(out=wv_sb[:, kt, :], in_=wva[:, kt, :])
        for kt in range(KT):
            dmae[kt % 2].dma_start(out=wq_sb[:, kt, :], in_=wqa[:, kt, :])
        for kt in range(KT):
            dmae[kt % 2].dma_start(
                out=x_sb[:, kt, QB : 2 * QB], in_=xT[:, kt, QB : 2 * QB]
            )
        for kt in range(KT):
            nc.scalar.dma_start(out=wk_sb[:, kt, :], in_=wka[:, kt, :])
        nc.scalar.dma_start(out=wqkb_sb, in_=wqkb[:])
        for blk in range(2, 4):
            for kt in range(KT):
                dma_rr(
                    x_sb[:, kt, blk * QB : (blk + 1) * QB],
                    xT[:, kt, blk * QB : (blk + 1) * QB],
                )
        nc.scalar.dma_start(out=wo_sb, in_=wo[:])

        # mask-bias constant: maskB[k, q] = FE_B if q >= k else MASK_FILL
        maskB = consts.tile([P, P], F32)
        nc.gpsimd.memset(maskB, FE_B)
        nc.gpsimd.affine_select(
            out=maskB,
            in_=maskB,
            compare_op=mybir.AluOpType.is_ge,
            fill=MASK_FILL,
            base=0,
            channel_multiplier=-1,
            pattern=[[1, P]],
        )
        # triu[k, q] = 1 iff q >= k (allowed), for masking ACT-exp'd diagonals
        from concourse.masks import make_upper_triangular

        triu_st = consts.tile([P, P], F32)
        make_upper_triangular(nc, triu_st, val=1.0, diag=True)
        triu = consts.tile([P, P], MMDT)
        nc.vector.tensor_copy(triu, triu_st)

        # ---- QKV projections (emitted interleaved with attention below).
        # q/k/v live in PER-BLOCK tiles so a later block's projection (write)
        # doesn't create a false whole-tile dependency against attention
        # units reading earlier blocks.
        qT_t = [qkv.tile([P, 2, QB], MMDT, name=f"qT{b}") for b in range(NU)]
        kT_t = [qkv.tile([P, 2, QB], MMDT, name=f"kT{b}") for b in range(NU)]
        v_t = [
            qkv.tile([P, KT, HPC * VW], MMDT, name=f"v{b}") for b in range(NU)
        ]

        def qk_proj(wi, w_sb, dst, m, blk):
            ps = mmps.tile([P, 2, QB], F32, tag="mm", name="ps")
            for kt in range(KT):
                nc.tensor.matmul(
                    ps[:, 0, :],
                    lhsT=w_sb[:, kt, m * P : (m + 1) * P],
                    rhs=x_sb[:, kt, blk * QB : (blk + 1) * QB],
                    start=(kt == 0),
                    stop=(kt == KT - 1),
                )
            nc.scalar.activation(
                out=dst[:, m, :], in_=ps[:, 0, :],
                func=IDENT,
                bias=wqkb_sb[:, 2 * wi + m : 2 * wi + m + 1],
            )

        def qk_blk(blk):
            for m in range(2):
                qk_proj(0, wq_sb, qT_t[blk], m, blk)
                qk_proj(1, wk_sb, kT_t[blk], m, blk)

        # v bias is folded into the host-side output bias, so v here is
        # bias-free; the denominator ones-columns are memset directly.
        def v_proj(tt):
            tag = f"o{tt % 2}{'ab'[(tt // 2) % 2]}"
            ps = aps.tile([P, QB], F32, tag=tag, name="vps")
            for kt in range(KT):
                nc.tensor.matmul(
                    ps[:, 0 : HPC * VW],
                    lhsT=x_sb[:, kt, tt * P : (tt + 1) * P],
                    rhs=wv_sb[:, kt, :],
                    start=(kt == 0),
                    stop=(kt == KT - 1),
                )
            vt = v_t[tt // 4]
            nc.vector.tensor_copy(vt[:, tt % 4, :], ps[:, 0 : HPC * VW])
            ones_cols = vt[:, tt % 4, :].rearrange(
                "p (h w) -> p h w", w=VW
            )[:, :, HD]
            nc.gpsimd.memset(ones_cols, 1.0)

        # ---- attention units
        # per-(qb, hp) normalized-output tiles: out_proj(qb) then only
        # depends on its own query block's normalization (whole-tile
        # dependency tracking would otherwise serialize the tail)
        attn_p = [
            [
                attn.tile([P, QB], MMDT, tag=f"attnp{qb}{hp}", name=f"attnp{qb}{hp}")
                for hp in range(2)
            ]
            for qb in range(NU)
        ]

        def emit_norm(hp, qb, opss):
            # normalization reads the AV psum directly; the bank frees at
            # the last read (the per-head multiply).  The reciprocal runs on
            # the denominator row DMA-reshaped to [128, 4] (wide in
            # partitions: single-partition reciprocal_approx_fast misbehaves
            # on HW, and a 64-row post-broadcast reciprocal wastes DVE).
            for i in (0, 1):
                rec = nrm.tile([VW, QB], F32, tag="rec", name="rec")
                nc.scalar.copy(rec[HD : HD + 1, :], opss[i][HD : HD + 1, :])
                recT = nrm.tile([P, QB // P], F32, tag=f"rT{i}", name=f"rT{i}")
                nc.sync.dma_start(out=recT, in_=rec[HD : HD + 1, :])
                recT2 = nrm.tile([P, QB // P], F32, tag=f"rU{i}", name=f"rU{i}")
                nc.vector.reciprocal_approx_fast(out=recT2, in_=recT)
                den0 = nrm.tile([1, QB], F32, tag=f"den{i}", name=f"den{i}")
                nc.sync.dma_start(out=den0, in_=recT2)
                bc = nrm.tile([HD, QB], F32, tag=f"bc{i}", name=f"bc{i}")
                nc.gpsimd.partition_broadcast(bc, den0)
                if i == 0:
                    nc.vector.tensor_mul(
                        attn_p[qb][hp][0:HD, :], opss[i][0:HD, :], bc
                    )
                else:
                    # odd head: normalize into a scratch at lanes 0-63,
                    # then DMA-hop to lanes 64-127 of the pair tile
                    odd = nrm.tile([HD, QB], MMDT, tag="odd", name="odd")
                    nc.vector.tensor_mul(odd, opss[i][0:HD, :], bc)
                    nc.sync.dma_start(out=attn_p[qb][hp][HD:P, :], in_=odd)

        def emit_unit(qb, hp, uidx, pending):
            pair = (2 * hp, 2 * hp + 1)
            qhs = [
                qT_t[qb][(h % 2) * HD : (h % 2) * HD + HD, h // 2, :]
                for h in pair
            ]
            st = "ab"[uidx % 2]
            opss = [
                aps.tile([P, QB], F32, tag=f"o{i}{st}", name=f"o{i}{st}")
                for i in range(2)
            ]
            nkt = (qb + 1) * (QB // P)
            for kt in range(nkt):
                off = max(0, kt * P - qb * QB)
                diag = kt * P >= qb * QB
                # scores for both heads, row-tiled concurrent, into one
                # [128, 2, 512] psum tile
                sps = mmps.tile([P, 2, QB], F32, tag="mm", name="sps")
                kTb = kT_t[kt // 4]
                for i in (0, 1):
                    h = pair[i]
                    nc.tensor.matmul(
                        sps[:, i, off:QB],
                        lhsT=kTb[
                            (h % 2) * HD : (h % 2) * HD + HD,
                            h // 2,
                            (kt % 4) * P : (kt % 4 + 1) * P,
                        ],
                        rhs=qhs[i][:, off:QB],
                        start=True,
                        stop=True,
                    )
                eT = etp.tile([P, 2, QB], MMDT, tag="eT", name="eT")
                # exp evacuation split BY HEAD across the two engines (ACT:
                # exact spline exp, DVE: Schraudolph): per-head latency is
                # ~half a tile so the one-step AV lag hides it, and each AV
                # matmul only waits on its own head's eT region.  The engine
                # per head alternates with kt parity.  On DVE-side diagonal
                # blocks the causal mask is fused into the evacuation (the
                # per-element bias constant); ACT-side diagonal blocks get
                # the triu multiply.
                for i in (0, 1):
                    on_act = (i + kt) % 2 == 0
                    if on_act:
                        nc.scalar.activation(
                            out=eT[:, i, off:QB], in_=sps[:, i, off:QB],
                            func=EXP, scale=1.0 / np.sqrt(HD),
                        )
                        if diag:
                            nc.vector.tensor_mul(
                                eT[:, i, off : off + P],
                                eT[:, i, off : off + P],
                                triu,
                            )
                    elif diag:
                        nc.vector.scalar_tensor_tensor(
                            out=eT[:, i, off : off + P].bitcast(I16),
                            in0=sps[:, i, off : off + P],
                            scalar=FE_A,
                            in1=maskB,
                            op0=mybir.AluOpType.mult,
                            op1=mybir.AluOpType.add,
                        )
                        if off + P < QB:
                            nc.vector.tensor_scalar(
                                out=eT[:, i, off + P : QB].bitcast(I16),
                                in0=sps[:, i, off + P : QB],
                                scalar1=FE_A, scalar2=FE_B,
                                op0=mybir.AluOpType.mult,
                                op1=mybir.AluOpType.add,
                            )
                    else:
                        nc.vector.tensor_scalar(
                            out=eT[:, i, off:QB].bitcast(I16),
                            in0=sps[:, i, off:QB],
                            scalar1=FE_A, scalar2=FE_B,
                            op0=mybir.AluOpType.mult,
                            op1=mybir.AluOpType.add,
                        )

                # emit the PREVIOUS step's AV matmuls now (one-step lag keeps
                # the in-order PE queue from stalling on this tile's exp)
                if pending[0] is not None:
                    pending[0][0]()
                    if pending[0][1]:
                        # defer the norm chain one unit (engine queues are
                        # strict FIFO: emitting it now would park ops that
                        # wait on the last AV in front of ready exp work)
                        norm_q.append(pending[0][2])
                    pending[0] = None
                if kt == 2 and norm_q:
                    norm_q.pop(0)()

                def av(eT=eT, off=off, kt=kt, opss=opss, pair=pair, nkt=nkt):
                    vt = v_t[kt // 4]
                    for i in (0, 1):
                        nc.tensor.matmul(
                            opss[i][0:VW, off:QB],
                            lhsT=vt[
                                :, kt % 4, pair[i] * VW : (pair[i] + 1) * VW
                            ],
                            rhs=eT[:, i, off:QB],
                            start=(kt == 0),
                            stop=(kt == nkt - 1),
                        )

                def norm(hp=hp, qb=qb, opss=opss):
                    emit_norm(hp, qb, opss)

                pending[0] = (av, kt == nkt - 1, norm)

        def flush(pending):
            if pending[0] is not None:
                pending[0][0]()
                if pending[0][1]:
                    norm_q.append(pending[0][2])
                pending[0] = None

        def flush_norms():
            while norm_q:
                norm_q.pop(0)()

        # out-projection for query rows qb*512 .. +512:
        # y[t, d] = attn_p0.T @ wo_p0 + attn_p1.T @ wo_p1  (K=128 each)
        def out_proj(qb, spread_y=None):
            for j in range(4):
                tt = 4 * qb + j
                ps = mmps.tile([P, 2, QB], F32, tag="mm", name="yps")
                for hp in range(2):
                    nc.tensor.matmul(
                        ps[:, 0, :],
                        lhsT=attn_p[qb][hp][:, j * P : (j + 1) * P],
                        rhs=wo_sb[:, hp, :],
                        start=(hp == 0),
                        stop=(hp == 1),
                    )
                yt = yevac.tile([P, D], BF16, tag="yt", name="yt")
                if tt % 2:
                    nc.scalar.copy(yt, ps[:, 0, :])
                else:
                    nc.vector.tensor_copy(yt, ps[:, 0, :])
                eng = spread_y[j % len(spread_y)] if spread_y else nc.sync
                eng.dma_start(out=y[tt * P : (tt + 1) * P, :], in_=yt)

        # ---- the interleaved emission schedule
        pending = [None]
        norm_q = []
        for tt in range(4):
            v_proj(tt)
        qk_blk(0)
        emit_unit(0, 0, 0, pending)
        emit_unit(0, 1, 1, pending)
        flush(pending)
        qk_blk(1)
        for tt in range(4, 8):
            v_proj(tt)
        emit_unit(1, 0, 2, pending)
        emit_unit(1, 1, 3, pending)
        flush(pending)
        qk_blk(2)
        for tt in range(8, 12):
            v_proj(tt)
        out_proj(0)
        emit_unit(2, 0, 4, pending)
        emit_unit(2, 1, 5, pending)
        flush(pending)
        qk_blk(3)
        for tt in range(12, 16):
            v_proj(tt)
        out_proj(1)
        emit_unit(3, 0, 6, pending)
        emit_unit(3, 1, 7, pending)
        flush(pending)
        # last unit's norm chain first (its ops land at the head of the
        # ACT/sync/gpsimd queues), then the out-projections with their y
        # DMAs on queues that won't block it
        flush_norms()
        out_proj(2, spread_y=[nc.scalar, nc.gpsimd])
        out_proj(3, spread_y=[nc.sync, nc.scalar, nc.gpsimd])

    nc.compile()
    return nc


def make_in_maps(x, W_qkv, b_qkv, W_out):
    x = np.asarray(x, np.float32)
    W_qkv = np.asarray(W_qkv, np.float32)
    b_qkv = np.asarray(b_qkv, np.float32)
    W_out = np.asarray(W_out, np.float32)
    in_maps = []
    for c in range(2 * B):
        b, g = divmod(c, 2)
        ch = g * HPC * HD
        wqa = W_qkv[:, ch : ch + 256].reshape(KT, P, 2 * P).transpose(1, 0, 2)
        wka = W_qkv[:, D + ch : D + ch + 256].reshape(KT, P, 2 * P).transpose(1, 0, 2)
        wqkb = np.concatenate(
            [
                b_qkv[ch : ch + 256].reshape(2, P).T,
                b_qkv[D + ch : D + ch + 256].reshape(2, P).T,
            ],
            axis=1,
        )  # [128, 4]: cols = q-m0, q-m1, k-m0, k-m1
        wva = np.zeros((D, HPC * VW), np.float32)
        wva3 = wva.reshape(D, HPC, VW)
        wva3[:, :, :HD] = W_qkv[:, 2 * D + ch : 2 * D + ch + 256].reshape(D, HPC, HD)
        wva = wva.reshape(KT, P, HPC * VW).transpose(1, 0, 2)
        wo = W_out[ch : ch + 256, :].reshape(2, P, D).transpose(1, 0, 2)
        xTc = np.ascontiguousarray(
            x[b].T.reshape(KT, P, T).transpose(1, 0, 2)
        )  # [P, KT, T]
        in_maps.append(
            {
                "xT": xTc.astype(_NP_MMDT),
                "wqa": np.ascontiguousarray(wqa).astype(_NP_MMDT),
                "wka": np.ascontiguousarray(wka).astype(_NP_MMDT),
                "wva": np.ascontiguousarray(wva).astype(_NP_MMDT),
                "wqkb": np.ascontiguousarray(wqkb, np.float32),
                "wo": np.ascontiguousarray(wo).astype(_NP_MMDT),
            }
        )
    return in_maps


def assemble(results, b_out, vbias_y):
    b_out = np.asarray(b_out, np.float32) + vbias_y
    out = np.empty((B, T, D), np.float32)
    for b in range(B):
        out[b] = (
            results[2 * b]["y"].astype(np.float32)
            + results[2 * b + 1]["y"].astype(np.float32)
            + b_out[None, :]
        )
    return out


_CACHE = {}


def kernel(x, W_qkv, b_qkv, W_out, b_out):
    if "nc" not in _CACHE:
        _CACHE["nc"] = build_bass()
    in_maps = make_in_maps(x, W_qkv, b_qkv, W_out)
    # v-bias contribution: softmax weights sum to 1, so b_v passes through
    # attention unchanged and lands as (b_v @ W_out) on every token.
    vbias_y = np.asarray(b_qkv, np.float32)[2 * D :] @ np.asarray(W_out, np.float32)
    res = run_bass_kernel_spmd(_CACHE["nc"], in_maps, list(range(2 * B)))
    return assemble(res.results, b_out, vbias_y)


# revision 49
# speedup vs baseline: 1.2646x; 1.2646x over previous
"""Multi-head causal self-attention on 8 Trainium2 NeuronCores.

Sharding: core c -> batch b = c // 2, heads 4*(c % 2) .. +4  (data parallel on
B, tensor parallel on heads).  Each core computes its 4 heads' attention for
its batch plus the partial out-projection; the host sums the two partials per
batch and adds b_out.

Per-core layout:
  xT   [128, 4, T]  x[b] transposed on host (bf16), partition-major
  qT/kT [128, 2, T] head-major: partitions = 2 heads x 64, 2 m-tiles
  v    [128, 16, 260] natural [T, hd] per head + a ones column (gives the
                    softmax denominator for free during the AV matmul)
  scores are computed transposed: sT[k, q] = kT.T @ q, the two heads of a
  pair run as row-tiled CONCURRENT matmuls (tile rows 0-63 / 64-127) into
  one [128, 2, 512] PSUM tile.

The whole kernel is emitted as one interleaved PE stream so the PE never
idles long enough for the HAM clock gate to re-throttle:
  v(0:4) qkv(b0) u(0,0) u(0,1) qkv(b1) v(4:8) u(1,0) u(1,1)
  qkv(b2) v(8:12) op(0) u(2,0) u(2,1) qkv(b3) v(12:16) op(1)
  u(3,0) u(3,1) op(2) op(3)
where u(qb,hp) is a 512-query attention unit and op(qb) the out-projection
rows qb*512..+512 (emitted once both hp units' normalized outputs exist).

Exp evacuation of score PSUM is column-split between ACT (exact exp) and
DVE (Schraudolph exp2 bit-trick: tensor_scalar -> int16 -> bf16 bitcast,
~3% per element, cancels in softmax).  Causal masking of diagonal blocks
is FUSED into the DVE evacuation: scalar_tensor_tensor adds a per-element
constant (FE_B on allowed entries, MASK_FILL on disallowed) so masked
entries bitcast to ~-3e-36 ~ -0.0 -- no separate mask multiply, and safe
whether the f32->i16 conversion saturates or wraps (it stays in range).

The AV matmuls lag the score matmuls by one key-tile step (the PE queue is
in-order; the lag hides the exp latency).  AV accumulator PSUM sets
alternate per unit.  Normalization reads the AV PSUM directly (frees the
bank at the last read): reciprocal of just the denominator row, DMA hop to
partition 0, gpsimd partition-broadcast, one multiply per head.
"""

import os
import sys
from contextlib import ExitStack

import numpy as np

for _p in ("/opt/trn_rl_repo", "/opt/pypackages"):
    if os.path.isdir(_p) and _p not in sys.path:
        sys.path.append(_p)

import concourse.bass as bass
from concourse import bacc
import concourse.mybir as mybir
import concourse.tile as tile
from concourse.bass_utils import run_bass_kernel_spmd


B, T, D = 4, 2048, 512
H, HD = 8, 64
HPC = 4  # heads per core
P = 128
KT = D // P  # k-tiles over the model dim
QB = 512  # query-unit width / psum bank width
NKT = T // P  # key tiles
NU = T // QB  # query blocks
VW = HD + 1  # v columns per head incl. the ones column

F32 = mybir.dt.float32
I16 = mybir.dt.int16
BF16 = mybir.dt.bfloat16
MMDT = BF16
EXP = mybir.ActivationFunctionType.Exp
IDENT = mybir.ActivationFunctionType.Identity

# fast-exp constants: exp(s/8) ~= bitcast_bf16(int16(s * FE_A + FE_B))
_LOG2E = 1.4426950408889634
FE_A = _LOG2E * 128.0 / 8.0
FE_B = 127.0 * 128.0 - 5.6
# masked entries: s*FE_A + MASK_FILL stays in [-32768, -31000] for |s|<=24,
# whose int16 bit patterns read as bf16 denormals ~ -3e-36 ~ zero.
MASK_FILL = -32214.0

# column where the ACT/DVE split of a full key-tile's exp evacuation sits
CA = 288  # ACT gets [0, CA), DVE gets [CA, 512)

NORM_OLD = os.environ.get("NORM_OLD", "0") == "1"
EXP_OLD = os.environ.get("EXP_OLD", "0") == "1"

try:
    import ml_dtypes
    _NP_MMDT = np.dtype(ml_dtypes.bfloat16)
except ImportError:
    _NP_MMDT = np.float32


def build_bass():
    nc = bacc.Bacc()
    xT = nc.declare_dram_parameter("xT", [P, KT, T], MMDT, isOutput=False)
    wqa = nc.declare_dram_parameter("wqa", [P, KT, 2 * P], MMDT, isOutput=False)
    wka = nc.declare_dram_parameter("wka", [P, KT, 2 * P], MMDT, isOutput=False)
    # q/k biases, laid out [channel % 128, channel // 128] for ACT bias APs
    wqkb = nc.declare_dram_parameter("wqkb", [P, 4], F32, isOutput=False)
    wva = nc.declare_dram_parameter("wva", [P, KT, HPC * VW], MMDT, isOutput=False)
    wo = nc.declare_dram_parameter("wo", [P, 2, D], MMDT, isOutput=False)
    y = nc.declare_dram_parameter("y", [T, D], BF16, isOutput=True)

    with tile.TileContext(nc) as tc, ExitStack() as ctx:
        consts = ctx.enter_context(tc.tile_pool(name="consts", bufs=1))
        qkv = ctx.enter_context(tc.tile_pool(name="qkv", bufs=1))
        attn = ctx.enter_context(tc.tile_pool(name="attn", bufs=1))
        etp = ctx.enter_context(tc.tile_pool(name="etp", bufs=4))
        nrm = ctx.enter_context(tc.tile_pool(name="nrm", bufs=3))
        yevac = ctx.enter_context(tc.tile_pool(name="yevac", bufs=3))
        # PSUM: "mm" 2 bufs x 4KB/partition (2 banks each) = 4 banks;
        # o{i}{s} 4 x [128,512]f32 (1 bank each) = 4 banks.
        mmps = ctx.enter_context(tc.tile_pool(name="mmps", bufs=2, space="PSUM"))
        aps = ctx.enter_context(tc.tile_pool(name="aps", bufs=1, space="PSUM"))

        # ---- SBUF destinations
        x_sb = consts.tile([P, KT, T], MMDT)
        wq_sb = consts.tile([P, KT, 2 * P], MMDT)
        wk_sb = consts.tile([P, KT, 2 * P], MMDT)
        wv_sb = consts.tile([P, KT, HPC * VW], MMDT)
        wqkb_sb = consts.tile([P, 4], F32)
        wo_sb = consts.tile([P, 2, D], MMDT)

        # ---- input DMA, ordered so the earliest compute's operands land
        # first; round-robin across the three DMA-capable engine queues but
        # keep the scalar engine light early (it starts evacuating PSUM soon)
        dmae = [nc.sync, nc.gpsimd, nc.scalar]
        rr = [0]

        def dma_rr(out, in_):
            dmae[rr[0] % 3].dma_start(out=out, in_=in_)
            rr[0] += 1

        # v projection runs first: it needs wv + x column block 0.  x lands
        # via sync+gpsimd queues; the scalar queue takes the weights (it has
        # no compute role until the first PSUM evacuations).
        for kt in range(KT):
            dmae[kt % 2].dma_start(
                out=x_sb[:, kt, 0:QB], in_=xT[:, kt, 0:QB]
            )
        for kt in range(KT):
            nc.scalar.dma_start(out=wv_sb[:, kt, :], in_=wva[:, kt, :])
        for kt in range(KT):
            dmae[kt % 2].dma_start(out=wq_sb[:, kt, :], in_=wqa[:, kt, :])
        for kt in range(KT):
            dmae[kt % 2].dma_start(
                out=x_sb[:, kt, QB : 2 * QB], in_=xT[:, kt, QB : 2 * QB]
            )
        for kt in range(KT):
            nc.scalar.dma_start(out=wk_sb[:, kt, :], in_=wka[:, kt, :])
        nc.scalar.dma_start(out=wqkb_sb, in_=wqkb[:])
        for blk in range(2, 4):
            for kt in range(KT):
                dma_rr(
                    x_sb[:, kt, blk * QB : (blk + 1) * QB],
                    xT[:, kt, blk * QB : (blk + 1) * QB],
                )
        nc.scalar.dma_start(out=wo_sb, in_=wo[:])

        # mask-bias constant: maskB[k, q] = FE_B if q >= k else MASK_FILL
        maskB = consts.tile([P, P], F32)
        nc.gpsimd.memset(maskB, FE_B)
        nc.gpsimd.affine_select(
            out=maskB,
            in_=maskB,
            compare_op=mybir.AluOpType.is_ge,
            fill=MASK_FILL,
            base=0,
            channel_multiplier=-1,
            pattern=[[1, P]],
        )
        # triu[k, q] = 1 iff q >= k (allowed), for masking ACT-exp'd diagonals
        from concourse.masks import make_upper_triangular

        triu_st = consts.tile([P, P], F32)
        make_upper_triangular(nc, triu_st, val=1.0, diag=True)
        triu = consts.tile([P, P], MMDT)
        nc.vector.tensor_copy(triu, triu_st)

        # ---- QKV projections (emitted interleaved with attention below).
        # q/k/v live in PER-BLOCK tiles so a later block's projection (write)
        # doesn't create a false whole-tile dependency against attention
        # units reading earlier blocks.
        qT_t = [qkv.tile([P, 2, QB], MMDT, name=f"qT{b}") for b in range(NU)]
        kT_t = [qkv.tile([P, 2, QB], MMDT, name=f"kT{b}") for b in range(NU)]
        v_t = [
            qkv.tile([P, KT, HPC * VW], MMDT, name=f"v{b}") for b in range(NU)
        ]

        def qk_proj(wi, w_sb, dst, m, blk):
            ps = mmps.tile([P, 2, QB], F32, tag="mm", name="ps")
            for kt in range(KT):
                nc.tensor.matmul(
                    ps[:, 0, :],
                    lhsT=w_sb[:, kt, m * P : (m + 1) * P],
                    rhs=x_sb[:, kt, blk * QB : (blk + 1) * QB],
                    start=(kt == 0),
                    stop=(kt == KT - 1),
                )
            nc.scalar.activation(
                out=dst[:, m, :], in_=ps[:, 0, :],
                func=IDENT,
                bias=wqkb_sb[:, 2 * wi + m : 2 * wi + m + 1],
            )

        def qk_blk(blk):
            for m in range(2):
                qk_proj(0, wq_sb, qT_t[blk], m, blk)
                qk_proj(1, wk_sb, kT_t[blk], m, blk)

        # v bias is folded into the host-side output bias, so v here is
        # bias-free; the denominator ones-columns are memset directly.
        def v_proj(tt):
            tag = f"o{tt % 2}{'ab'[(tt // 2) % 2]}"
            ps = aps.tile([P, QB], F32, tag=tag, name="vps")
            for kt in range(KT):
                nc.tensor.matmul(
                    ps[:, 0 : HPC * VW],
                    lhsT=x_sb[:, kt, tt * P : (tt + 1) * P],
                    rhs=wv_sb[:, kt, :],
                    start=(kt == 0),
                    stop=(kt == KT - 1),
                )
            vt = v_t[tt // 4]
            nc.vector.tensor_copy(vt[:, tt % 4, :], ps[:, 0 : HPC * VW])
            ones_cols = vt[:, tt % 4, :].rearrange(
                "p (h w) -> p h w", w=VW
            )[:, :, HD]
            nc.gpsimd.memset(ones_cols, 1.0)

        # ---- attention units
        # per-(qb, hp) normalized-output tiles: out_proj(qb) then only
        # depends on its own query block's normalization (whole-tile
        # dependency tracking would otherwise serialize the tail)
        attn_p = [
            [
                attn.tile([P, QB], MMDT, tag=f"attnp{qb}{hp}", name=f"attnp{qb}{hp}")
                for hp in range(2)
            ]
            for qb in range(NU)
        ]

        def emit_norm(hp, qb, opss):
            # normalization reads the AV psum directly; the bank frees at
            # the last read (the per-head multiply).  The reciprocal runs on
            # the denominator row DMA-reshaped to [128, 4] (wide in
            # partitions: single-partition reciprocal_approx_fast misbehaves
            # on HW, and a 64-row post-broadcast reciprocal wastes DVE).
            for i in (0, 1):
                rec = nrm.tile([VW, QB], F32, tag="rec", name="rec")
                nc.scalar.copy(rec[HD : HD + 1, :], opss[i][HD : HD + 1, :])
                recT = nrm.tile([P, QB // P], F32, tag=f"rT{i}", name=f"rT{i}")
                nc.sync.dma_start(out=recT, in_=rec[HD : HD + 1, :])
                recT2 = nrm.tile([P, QB // P], F32, tag=f"rU{i}", name=f"rU{i}")
                nc.vector.reciprocal_approx_fast(out=recT2, in_=recT)
                den0 = nrm.tile([1, QB], F32, tag=f"den{i}", name=f"den{i}")
                nc.sync.dma_start(out=den0, in_=recT2)
                bc = nrm.tile([HD, QB], F32, tag=f"bc{i}", name=f"bc{i}")
                nc.gpsimd.partition_broadcast(bc, den0)
                if i == 0:
                    nc.vector.tensor_mul(
                        attn_p[qb][hp][0:HD, :], opss[i][0:HD, :], bc
                    )
                else:
                    # odd head: normalize into a scratch at lanes 0-63,
                    # then DMA-hop to lanes 64-127 of the pair tile
                    odd = nrm.tile([HD, QB], MMDT, tag="odd", name="odd")
                    nc.vector.tensor_mul(odd, opss[i][0:HD, :], bc)
                    nc.sync.dma_start(out=attn_p[qb][hp][HD:P, :], in_=odd)

        def emit_unit(qb, hp, uidx, pending):
            pair = (2 * hp, 2 * hp + 1)
            qhs = [
                qT_t[qb][(h % 2) * HD : (h % 2) * HD + HD, h // 2, :]
                for h in pair
            ]
            st = "ab"[uidx % 2]
            opss = [
                aps.tile([P, QB], F32, tag=f"o{i}{st}", name=f"o{i}{st}")
                for i in range(2)
            ]
            nkt = (qb + 1) * (QB // P)
            for kt in range(nkt):
                off = max(0, kt * P - qb * QB)
                diag = kt * P >= qb * QB
                # scores for both heads, row-tiled concurrent, into one
                # [128, 2, 512] psum tile
                sps = mmps.tile([P, 2, QB], F32, tag="mm", name="sps")
                kTb = kT_t[kt // 4]
                for i in (0, 1):
                    h = pair[i]
                    nc.tensor.matmul(
                        sps[:, i, off:QB],
                        lhsT=kTb[
                            (h % 2) * HD : (h % 2) * HD + HD,
                            h // 2,
                            (kt % 4) * P : (kt % 4 + 1) * P,
                        ],
                        rhs=qhs[i][:, off:QB],
                        start=True,
                        stop=True,
                    )
                eT = etp.tile([P, 2, QB], MMDT, tag="eT", name="eT")
                # whole-tile exp evacuation, alternating ACT (exact spline
                # exp) / DVE (Schraudolph) by key-tile parity; one big
                # instruction per tile (the engines have a large fixed cost
                # per instruction).  The AV lag of TWO key tiles (below)
                # hides the full exp latency.
                if kt % 2 == 0:
                    nc.scalar.activation(
                        out=eT[:, :, off:QB], in_=sps[:, :, off:QB],
                        func=EXP, scale=1.0 / np.sqrt(HD),
                    )
                    if diag:
                        for i in (0, 1):
                            nc.vector.tensor_mul(
                                eT[:, i, off : off + P],
                                eT[:, i, off : off + P],
                                triu,
                            )
                else:
                    if diag:
                        for i in (0, 1):
                            nc.vector.scalar_tensor_tensor(
                                out=eT[:, i, off : off + P].bitcast(I16),
                                in0=sps[:, i, off : off + P],
                                scalar=FE_A,
                                in1=maskB,
                                op0=mybir.AluOpType.mult,
                                op1=mybir.AluOpType.add,
                            )
                        if off + P < QB:
                            nc.vector.tensor_scalar(
                                out=eT[:, :, off + P : QB].bitcast(I16),
                                in0=sps[:, :, off + P : QB],
                                scalar1=FE_A, scalar2=FE_B,
                                op0=mybir.AluOpType.mult,
                                op1=mybir.AluOpType.add,
                            )
                    else:
                        nc.vector.tensor_scalar(
                            out=eT[:, :, off:QB].bitcast(I16),
                            in0=sps[:, :, off:QB],
                            scalar1=FE_A, scalar2=FE_B,
                            op0=mybir.AluOpType.mult,
                            op1=mybir.AluOpType.add,
                        )

                # emit the AV matmuls lagging TWO key-tile steps (the
                # in-order PE queue then never stalls on exp latency; the
                # pair of score tiles ahead live in the two sps buffers)
                if len(pending) >= 2:
                    av_, last_, norm_ = pending.pop(0)
                    av_()
                    if last_:
                        # defer the norm chain (engine queues are strict
                        # FIFO: emitting it now would park ops that wait on
                        # the last AV in front of ready exp work)
                        norm_q.append(norm_)
                if kt == 3 and norm_q:
                    norm_q.pop(0)()

                def av(eT=eT, off=off, kt=kt, opss=opss, pair=pair, nkt=nkt):
                    vt = v_t[kt // 4]
                    for i in (0, 1):
                        nc.tensor.matmul(
                            opss[i][0:VW, off:QB],
                            lhsT=vt[
                                :, kt % 4, pair[i] * VW : (pair[i] + 1) * VW
                            ],
                            rhs=eT[:, i, off:QB],
                            start=(kt == 0),
                            stop=(kt == nkt - 1),
                        )

                def norm(hp=hp, qb=qb, opss=opss):
                    emit_norm(hp, qb, opss)

                pending.append((av, kt == nkt - 1, norm))

        def flush(pending):
            while pending:
                av_, last_, norm_ = pending.pop(0)
                av_()
                if last_:
                    norm_q.append(norm_)

        def flush_norms():
            while norm_q:
                norm_q.pop(0)()

        # out-projection for query rows qb*512 .. +512:
        # y[t, d] = attn_p0.T @ wo_p0 + attn_p1.T @ wo_p1  (K=128 each)
        def out_proj(qb, spread_y=None):
            for j in range(4):
                tt = 4 * qb + j
                ps = mmps.tile([P, 2, QB], F32, tag="mm", name="yps")
                for hp in range(2):
                    nc.tensor.matmul(
                        ps[:, 0, :],
                        lhsT=attn_p[qb][hp][:, j * P : (j + 1) * P],
                        rhs=wo_sb[:, hp, :],
                        start=(hp == 0),
                        stop=(hp == 1),
                    )
                yt = yevac.tile([P, D], BF16, tag="yt", name="yt")
                if tt % 2:
                    nc.scalar.copy(yt, ps[:, 0, :])
                else:
                    nc.vector.tensor_copy(yt, ps[:, 0, :])
                eng = spread_y[j % len(spread_y)] if spread_y else nc.sync
                eng.dma_start(out=y[tt * P : (tt + 1) * P, :], in_=yt)

        # ---- the interleaved emission schedule
        pending = []
        norm_q = []
        for tt in range(4):
            v_proj(tt)
        qk_blk(0)
        emit_unit(0, 0, 0, pending)
        emit_unit(0, 1, 1, pending)
        flush(pending)
        qk_blk(1)
        for tt in range(4, 8):
            v_proj(tt)
        emit_unit(1, 0, 2, pending)
        emit_unit(1, 1, 3, pending)
        flush(pending)
        qk_blk(2)
        for tt in range(8, 12):
            v_proj(tt)
        out_proj(0)
        emit_unit(2, 0, 4, pending)
        emit_unit(2, 1, 5, pending)
        flush(pending)
        qk_blk(3)
        for tt in range(12, 16):
            v_proj(tt)
        out_proj(1)
        emit_unit(3, 0, 6, pending)
        emit_unit(3, 1, 7, pending)
        flush(pending)
        # last unit's norm chain first (its ops land at the head of the
        # ACT/sync/gpsimd queues), then the out-projections with their y
        # DMAs on queues that won't block it
        flush_norms()
        out_proj(2, spread_y=[nc.scalar, nc.gpsimd])
        out_proj(3, spread_y=[nc.sync, nc.scalar, nc.gpsimd])

    nc.compile()
    return nc


def make_in_maps(x, W_qkv, b_qkv, W_out):
    x = np.asarray(x, np.float32)
    W_qkv = np.asarray(W_qkv, np.float32)
    b_qkv = np.asarray(b_qkv, np.float32)
    W_out = np.asarray(W_out, np.float32)
    in_maps = []
    for c in range(2 * B):
        b, g = divmod(c, 2)
        ch = g * HPC * HD
        wqa = W_qkv[:, ch : ch + 256].reshape(KT, P, 2 * P).transpose(1, 0, 2)
        wka = W_qkv[:, D + ch : D + ch + 256].reshape(KT, P, 2 * P).transpose(1, 0, 2)
        wqkb = np.concatenate(
            [
                b_qkv[ch : ch + 256].reshape(2, P).T,
                b_qkv[D + ch : D + ch + 256].reshape(2, P).T,
            ],
            axis=1,
        )  # [128, 4]: cols = q-m0, q-m1, k-m0, k-m1
        wva = np.zeros((D, HPC * VW), np.float32)
        wva3 = wva.reshape(D, HPC, VW)
        wva3[:, :, :HD] = W_qkv[:, 2 * D + ch : 2 * D + ch + 256].reshape(D, HPC, HD)
        wva = wva.reshape(KT, P, HPC * VW).transpose(1, 0, 2)
        wo = W_out[ch : ch + 256, :].reshape(2, P, D).transpose(1, 0, 2)
        xTc = np.ascontiguousarray(
            x[b].T.reshape(KT, P, T).transpose(1, 0, 2)
        )  # [P, KT, T]
        in_maps.append(
            {
                "xT": xTc.astype(_NP_MMDT),
                "wqa": np.ascontiguousarray(wqa).astype(_NP_MMDT),
                "wka": np.ascontiguousarray(wka).astype(_NP_MMDT),
                "wva": np.ascontiguousarray(wva).astype(_NP_MMDT),
                "wqkb": np.ascontiguousarray(wqkb, np.float32),
                "wo": np.ascontiguousarray(wo).astype(_NP_MMDT),
            }
        )
    return in_maps


def assemble(results, b_out, vbias_y):
    b_out = np.asarray(b_out, np.float32) + vbias_y
    out = np.empty((B, T, D), np.float32)
    for b in range(B):
        out[b] = (
            results[2 * b]["y"].astype(np.float32)
            + results[2 * b + 1]["y"].astype(np.float32)
            + b_out[None, :]
        )
    return out


_CACHE = {}


def kernel(x, W_qkv, b_qkv, W_out, b_out):
    if "nc" not in _CACHE:
        _CACHE["nc"] = build_bass()
    in_maps = make_in_maps(x, W_qkv, b_qkv, W_out)
    # v-bias contribution: softmax weights sum to 1, so b_v passes through
    # attention unchanged and lands as (b_v @ W_out) on every token.
    vbias_y = np.asarray(b_qkv, np.float32)[2 * D :] @ np.asarray(W_out, np.float32)
    res = run_bass_kernel_spmd(_CACHE["nc"], in_maps, list(range(2 * B)))
    return assemble(res.results, b_out, vbias_y)
